# revision 8
# baseline (speedup 1.0000x reference)
"""Trainium2 Bass kernel for nn_Attention_24781961298297.

Math: scores[b,i,j] = (q_term[b,i] + k_term[b,j]) / sqrt(A).  Softmax over j
subtracts the row max, and q_term[b,i] is constant along j, so it cancels
exactly -- the attention weights are independent of i (and of the whole
decoder/q branch).  The output is one [A] vector per batch element,
broadcast over all Ld rows:

    kt[b,j] = relu(enc[b,j] @ Wk) @ (Pu @ pv)      (biases are zero)
    w[b]    = softmax(kt[b] / sqrt(A))
    row[b]  = w[b] @ relu(enc[b] @ Wv)
    out[b,i,:] = row[b]  for all i

Sharding: pure data-parallel over batch B=8 across the 8 cores (one batch
element per core, no collectives).

Implementation notes (v2, tuned from the 36.3us baseline trace):
  * enc is shipped fp8e4m3 in a host-prepared *piece-major* layout:
    for each 512-token piece, a [128, NDC*512] block whose per-partition
    row is that piece's 4 DE-subtiles concatenated -> every DMA
    descriptor moves a contiguous 2 KB run (the baseline's token-sliced
    pieces produced 512 B descriptors and measured only ~88 GB/s).
    Pieces alternate between the two HWDGE rings (sync + scalar).
  * Projections use fp8 DoubleRow matmuls (K=256 per instruction).
  * The softmax weighting + accumulation is one DVE scalar_tensor_tensor
    per chunk (v_sb * wb with accum_out); V-relu is on ACT, K-relu on
    DVE (biases are zero so relu is a plain max).
  * kt uses an M=1 matmul (lhsT = u column) -> [1, 512] PSUM rows.
  * The per-chunk PE program is software-pipelined
        KP_t, VP_t, ktp_{t-1}, wb_{t-2}
    so no PE instruction ever waits on the ACT/DVE results of its own
    chunk.  PSUM: kps(1) + vps(3) + ktp(1) + wb(2) + warm(1) = 8 banks.
  * A junk-matmul burst at kernel start keeps the PE busy so the HAM
    clock gate (1.2 -> 2.4 GHz) opens early.
"""

import numpy as np
import ml_dtypes

import concourse.bass as bass
import concourse.bacc as bacc
import concourse.tile as tile
from concourse import mybir
from concourse.bass_utils import run_bass_kernel_spmd

B, LE, LD = 8, 4096, 4096
DE, DD, A = 512, 512, 128

SZ = 512                 # tokens per chunk / DMA piece
NT = LE // SZ            # 8 chunks
NDC = DE // 128          # 4 DE subtiles
USE_DR = True            # fp8 DoubleRow projections

INV_SQRT_A = float(1.0 / np.sqrt(np.float32(A)))

F32 = mybir.dt.float32
BF16 = mybir.dt.bfloat16
FP8 = mybir.dt.float8e4
Relu = mybir.ActivationFunctionType.Relu
Exp = mybir.ActivationFunctionType.Exp
AX = mybir.AxisListType.X
ADD = mybir.AluOpType.add
MAX = mybir.AluOpType.max
MULT = mybir.AluOpType.mult
DR = mybir.MatmulPerfMode.DoubleRow

W_DT = FP8 if USE_DR else BF16
N_WARM = 6


def build_nc() -> bass.Bass:
    nc = bacc.Bacc()

    encp = nc.declare_dram_parameter("encp", [NT * 128, NDC * SZ], FP8,
                                     isOutput=False)
    wkv = nc.declare_dram_parameter("wkv", [128, NDC * 2 * A], W_DT,
                                    isOutput=False)
    u_pad = nc.declare_dram_parameter("u_pad", [A, 128], BF16, isOutput=False)
    out = nc.declare_dram_parameter("out", [A, 128], F32, isOutput=True)

    with tile.TileContext(nc) as tc:
        with (
            tc.tile_pool(name="consts", bufs=1) as consts,
            tc.tile_pool(name="encpool", bufs=1) as encpool,
            tc.tile_pool(name="kvp", bufs=2) as kvp,
            tc.tile_pool(name="smallp", bufs=1) as smallp,
            tc.tile_pool(name="work", bufs=2) as work,
            tc.tile_pool(name="ps_k", bufs=1, space="PSUM") as ps_k,
            tc.tile_pool(name="ps_v", bufs=2, space="PSUM") as ps_v,
            tc.tile_pool(name="ps_kt", bufs=1, space="PSUM") as ps_kt,
            tc.tile_pool(name="ps_wb", bufs=2, space="PSUM") as ps_wb,
            tc.tile_pool(name="ps_warm", bufs=1, space="PSUM") as ps_warm,
        ):
            # ---- constants on the ACT HWDGE ring (land before piece 0)
            up_sb = consts.tile([A, 128], BF16, tag="up")
            nc.scalar.dma_start(out=up_sb, in_=u_pad[:, :])
            wkv_sb = consts.tile([128, NDC, 2 * A], W_DT, tag="wkv")
            nc.scalar.dma_start(
                out=wkv_sb, in_=wkv.rearrange("p (c a) -> p c a", c=NDC))

            # ---- encoder pieces: [128, NDC*SZ] blocks, 2 KB/partition
            #      contiguous, alternating sync/scalar HWDGE rings.
            enc2 = encpool.tile([128, NT, NDC, SZ], FP8, tag="enc2")
            encr = encp.rearrange("(t p) f -> t p f", p=128)
            for t in range(NT):
                eng = nc.sync if t % 2 == 0 else nc.scalar
                eng.dma_start(
                    out=enc2[:, t, :, :],
                    in_=encr[t].rearrange("p (c j) -> p c j", c=NDC),
                )

            # ---- tiny SBUF constants + PE warm-up burst
            ones1 = consts.tile([1, 128], BF16, tag="ones1")
            nc.gpsimd.memset(ones1, 1.0)
            wtile = consts.tile([1, 512], BF16, tag="wtile")
            nc.gpsimd.memset(wtile, 0.5)
            for _ in range(N_WARM):
                warm_ps = ps_warm.tile([128, 512], F32, tag="warm")
                nc.tensor.matmul(warm_ps, lhsT=ones1, rhs=wtile,
                                 start=True, stop=True)

            e_sb = smallp.tile([1, LE], BF16, tag="e")
            ssum = smallp.tile([1, NT], F32, tag="ssum")
            partial = smallp.tile([A, NT], F32, tag="partial")
            out_pad = smallp.tile([A, 128], F32, tag="out_pad")
            nc.gpsimd.memset(out_pad, 0.0)

            kps_t = {}   # PSUM K-projection per chunk
            vps_t = {}   # PSUM V-projection per chunk
            ktp_t = {}   # PSUM [1, SZ] logits per chunk
            kT_t = {}    # SBUF relu'd K per chunk
            vT_t = {}    # SBUF relu'd V per chunk

            def emit_proj(t, half):
                """K (half=0) or V (half=1) projection for chunk t."""
                ps = (ps_k if half == 0 else ps_v).tile(
                    [128, SZ], F32, tag="kps" if half == 0 else "vps")
                lo, hi = (0, A) if half == 0 else (A, 2 * A)
                if USE_DR:
                    for c in range(0, NDC, 2):
                        nc.tensor.matmul(
                            ps, lhsT=wkv_sb[:, c:c + 2, lo:hi],
                            rhs=enc2[:, t, c:c + 2, :],
                            start=(c == 0), stop=(c == NDC - 2),
                            perf_mode=DR,
                        )
                else:
                    for c in range(NDC):
                        nc.tensor.matmul(
                            ps, lhsT=wkv_sb[:, c, lo:hi],
                            rhs=enc2[:, t, c, :],
                            start=(c == 0), stop=(c == NDC - 1),
                        )
                return ps

            def emit_relu_k(t):
                kT = kvp.tile([A, SZ], BF16, tag="kT")
                nc.vector.tensor_scalar(out=kT, in0=kps_t[t],
                                        scalar1=0.0, scalar2=None,
                                        op0=MAX)
                kT_t[t] = kT
                del kps_t[t]

            def emit_relu_v(t):
                vT = kvp.tile([A, SZ], BF16, tag="vT", bufs=3)
                nc.scalar.activation(out=vT, in_=vps_t[t], func=Relu,
                                     bias=0.0, scale=1.0)
                vT_t[t] = vT
                del vps_t[t]

            def emit_kt_exp(t):
                ktp = ps_kt.tile([1, SZ], F32, tag="ktp")
                nc.tensor.matmul(ktp, lhsT=up_sb[:, 0:1], rhs=kT_t[t],
                                 start=True, stop=True)
                ktp_t[t] = ktp
                del kT_t[t]
                nc.scalar.activation(
                    out=e_sb[0:1, t * SZ:(t + 1) * SZ], in_=ktp,
                    func=Exp, bias=0.0, scale=1.0,
                    accum_out=ssum[:, t:t + 1])

            def emit_wb_stt(t):
                wb = ps_wb.tile([128, SZ], F32, tag="wb")
                nc.tensor.matmul(wb, lhsT=ones1,
                                 rhs=e_sb[0:1, t * SZ:(t + 1) * SZ],
                                 start=True, stop=True)
                # v * softmax-weight with accumulated row-sum (one DVE op)
                prod = work.tile([A, SZ], BF16, tag="prod")
                nc.vector.scalar_tensor_tensor(
                    out=prod, in0=vT_t[t], scalar=0.0, in1=wb,
                    op0=mybir.AluOpType.bypass, op1=MULT,
                    accum_out=partial[:, t:t + 1])
                del vT_t[t]

            for t in range(NT):
                kps_t[t] = emit_proj(t, 0)
                emit_relu_k(t)
                vps_t[t] = emit_proj(t, 1)
                emit_relu_v(t)
                if t >= 1:
                    emit_kt_exp(t - 1)
                if t >= 2:
                    emit_wb_stt(t - 2)
            emit_kt_exp(NT - 1)
            emit_wb_stt(NT - 2)
            emit_wb_stt(NT - 1)

            # ---- unnormalized row + S; host divides and broadcasts.
            nc.vector.reduce_sum(out=out_pad[0:1, 1:2], in_=ssum, axis=AX,
                                 op=ADD)
            nc.vector.reduce_sum(out=out_pad[:, 0:1], in_=partial, axis=AX,
                                 op=ADD)
            nc.sync.dma_start(out=out[:, :], in_=out_pad)

    nc.finalize()
    return nc


def make_in_maps(inputs) -> list[dict]:
    f8 = ml_dtypes.float8_e4m3
    bf16 = ml_dtypes.bfloat16
    enc = np.asarray(inputs["encoder_outputs"], dtype=np.float32)
    Wk = np.asarray(inputs["Wk"], dtype=np.float32)
    Wv = np.asarray(inputs["Wv"], dtype=np.float32)
    Pu = np.asarray(inputs["Pu"], dtype=np.float32)
    pv = np.asarray(inputs["pv"], dtype=np.float32)

    u = (Pu @ pv).astype(np.float32) * INV_SQRT_A          # [A, 1]
    u_pad = np.zeros((A, 128), np.float32)
    u_pad[:, 0:1] = u
    u_pad = u_pad.astype(bf16)

    # [DE, 2A] -> [128 partitions, NDC * 2A] (c-major per partition)
    wkv = np.concatenate([Wk, Wv], axis=1)                 # [DE, 2A]
    wkv = np.ascontiguousarray(
        wkv.reshape(NDC, 128, 2 * A).transpose(1, 0, 2).reshape(128, -1)
    ).astype(f8 if USE_DR else bf16)

    maps = []
    for b in range(B):
        encT = np.ascontiguousarray(enc[b].T).astype(f8)   # [DE, LE]
        # piece-major: [t, p, c, j] with (c, j) contiguous per partition
        ep = np.ascontiguousarray(
            encT.reshape(NDC, 128, NT, SZ).transpose(2, 1, 0, 3)
            .reshape(NT * 128, NDC * SZ))
        maps.append({
            "encp": ep,
            "wkv": wkv,
            "u_pad": u_pad,
        })
    return maps


_NC_CACHE = None


def kernel(**inputs) -> np.ndarray:
    global _NC_CACHE
    in_maps = make_in_maps(inputs)
    if _NC_CACHE is None:
        _NC_CACHE = build_nc()
    res = run_bass_kernel_spmd(_NC_CACHE, in_maps, core_ids=list(range(B)))
    rows = []
    for b in range(B):
        o = np.asarray(res.results[b]["out"], dtype=np.float32)
        rows.append(o[:, 0] / o[0, 1])
    rows = np.stack(rows)                          # [B, A]
    return np.ascontiguousarray(
        np.broadcast_to(rows[:, None, :], (B, LD, A)).astype(np.float32)
    )
